# revision 1
# baseline (speedup 1.0000x reference)
"""Fused Fourier-block kernel for TRN2 (8 NeuronCores, data-parallel).

Reference computation (per token, C=1024, H=4096):
    h  = LN1(x)
    f  = real(FFT_C(h)) = h @ COS            (COS[n,k] = cos(2*pi*n*k/C))
    x2 = x + LNf(f)
    h2 = LN2(x2)
    m  = gelu_exact(h2 @ w1 + b1)
    out = x2 + m @ w2 + b2

Strategy: shard the 4*2048 = 8192 tokens over 8 cores (1024 tokens each).
All device math is done with activations CHANNEL-MAJOR ([channel, token]),
so every matmul consumes weights in their natural [in, out] layout and
chains without any device-side transposes (the host transposes each x shard
on the way in and the output shard on the way out).  LayerNorm reductions
over the channel (partition) dim are done on the TensorEngine as
ones-matmuls whose [128, T] PSUM output broadcasts the per-token sums to
every partition.  Matmul dtypes: fp32r (TF32-like, 1 cycle/row) for the
residual-path stats, fp16 for the three big matmuls (weights cast on host).
"""

from contextlib import ExitStack

import numpy as np

import concourse.bacc as bacc
import concourse.mybir as mybir
import concourse.tile as tile
from concourse.bass_utils import run_bass_kernel_spmd

AF = mybir.ActivationFunctionType
ALU = mybir.AluOpType

P = 128          # SBUF partitions
C = 1024         # channel dim
H = 4096         # MLP hidden dim
KO = C // P      # 8 channel chunks
HO = H // P      # 32 hidden chunks
TOK = 1024       # tokens per core
TT = 512         # token tile (matmul moving dim)
NT = TOK // TT   # 2 token tiles per core
N_CORES = 8
EPS = 1e-5

F32 = mybir.dt.float32
F32R = mybir.dt.float32r
F16 = mybir.dt.float16

# packed param columns (each [1024] vector becomes [128, 8] partition-major)
_PCOLS = {
    "ln1_g": 0, "ln1_b": 8, "lnf_g": 16, "lnf_b": 24,
    "ln2_g": 32, "ln2_b": 40, "b2": 48,
}
_B1_COL = 56  # b1 occupies cols 56..88
_GCS_COL = 88   # colsum(g*COS) for the 5 direct FFT chunks
_BFX_COL = 93   # (ln1_b @ COS) for the 5 direct FFT chunks
_G0_COL = 98    # ln1_g[0] replicated
_B0_COL = 99    # ln1_b[0] replicated
_PWIDTH = 100


def _build_nc():
    nc = bacc.Bacc()

    xT16 = nc.declare_dram_parameter("xT16", [C, TOK], F16, isOutput=False)
    fcos = nc.declare_dram_parameter("fcos", [C, 5 * P], F16, isOutput=False)
    w1b = nc.declare_dram_parameter("w1b", [HO, P, KO, P], F16, isOutput=False)
    w2b = nc.declare_dram_parameter("w2b", [KO, P, HO, P], F16, isOutput=False)
    mir = nc.declare_dram_parameter("mir", [2, P, P], F16, isOutput=False)
    params = nc.declare_dram_parameter("params", [P, _PWIDTH], F32, isOutput=False)
    outT = nc.declare_dram_parameter("outT", [C, TOK], F32R, isOutput=True)

    xT16_r = xT16.rearrange("(ko kp) t -> kp ko t", kp=P)
    fcos_r = fcos.rearrange("(ko kp) m -> kp ko m", kp=P)
    outT_r = outT.rearrange("(co cp) t -> cp co t", cp=P)

    with tile.TileContext(nc) as tc, ExitStack() as ctx:
        persist = ctx.enter_context(tc.tile_pool(name="persist", bufs=1))
        tmp = ctx.enter_context(tc.tile_pool(name="tmp", bufs=3))
        stat = ctx.enter_context(tc.tile_pool(name="stat", bufs=3))
        outp = ctx.enter_context(tc.tile_pool(name="outp", bufs=2))

        # ---------- constants ----------
        ones_h = persist.tile([P, P], F16)
        nc.vector.memset(ones_h, 1.0)
        eps_sb = persist.tile([P, 1], F32)
        nc.vector.memset(eps_sb, EPS)

        par_sb = persist.tile([P, _PWIDTH], F32)

        def pcol(name, k):
            c0 = _PCOLS[name] + k
            return par_sb[:, c0 : c0 + 1]

        # activations that live across both phases
        x2_sb = [persist.tile([P, KO, TT], F16, name=f"x2{t}") for t in range(NT)]
        h2_sb = [persist.tile([P, KO, TT], F16, name=f"h2{t}") for t in range(NT)]

        def ln_stats(src, ones, ps_s, ps_q, mu_bcast_src=None):
            """src: [P, KO, TT] fp16 tile. Returns (mu16, rstd16) [P, TT] fp16
            broadcast across all partitions. If mu_bcast_src is given (a
            [1, TT] AP already equal to the mean), broadcast it with a single
            K=1 matmul instead of the 8-matmul sum reduction."""
            psum_s = ps_s.tile([P, TT], F32, tag="ps_s")
            psum_q = ps_q.tile([P, TT], F32, tag="ps_q")
            if mu_bcast_src is not None:
                nc.tensor.matmul(
                    psum_s, lhsT=ones[0:1, :], rhs=mu_bcast_src,
                    start=True, stop=True,
                )
            else:
                for k in range(KO):
                    nc.tensor.matmul(
                        psum_s, lhsT=ones, rhs=src[:, k, :],
                        start=(k == 0), stop=(k == KO - 1),
                    )
            for k in range(KO):
                sq = tmp.tile([P, TT], F16, tag="sq")
                nc.vector.tensor_mul(sq, src[:, k, :], src[:, k, :])
                nc.tensor.matmul(
                    psum_q, lhsT=ones, rhs=sq,
                    start=(k == 0), stop=(k == KO - 1),
                )
            mu_scale = 1.0 if mu_bcast_src is not None else 1.0 / C
            mu16 = stat.tile([P, TT], F16, tag="mu")
            nc.scalar.activation(mu16, psum_s, AF.Copy, scale=mu_scale)
            musq = stat.tile([P, TT], F32, tag="musq")
            nc.scalar.activation(musq, psum_s, AF.Square, scale=mu_scale)
            var = stat.tile([P, TT], F32, tag="var")
            nc.scalar.activation(var, psum_q, AF.Copy, scale=1.0 / C)
            nc.vector.tensor_tensor(var, var, musq, ALU.subtract)
            nc.scalar.activation(var, var, AF.Sqrt, bias=eps_sb)
            rstd = stat.tile([P, TT], F32, tag="rstd")
            nc.vector.reciprocal_approx_fast(rstd, var)
            rstd16 = stat.tile([P, TT], F16, tag="rstd16")
            nc.vector.tensor_copy(rstd16, rstd)
            return mu16, rstd16

        def ln_apply_chunk(src, mu16, rstd16, gname, bname, dst, k):
            xc = tmp.tile([P, TT], F16, tag="xc")
            nc.vector.tensor_tensor(xc, src[:, k, :], mu16, ALU.subtract)
            nc.vector.tensor_tensor(xc, xc, rstd16, ALU.mult)
            nc.vector.tensor_scalar(
                dst[:, k, :], xc, pcol(gname, k), pcol(bname, k),
                ALU.mult, ALU.add,
            )

        def ln_apply(src, mu16, rstd16, gname, bname, dst):
            """dst[:, k, :] = (src[:, k, :] - mu) * rstd * g[k] + b[k]"""
            for k in range(KO):
                ln_apply_chunk(src, mu16, rstd16, gname, bname, dst, k)

        # ===== software pipeline across the two token tiles ================
        # PE-order: phase1(t0) | stats1(t1) | MLP1(t0) | FFT..LN2(t1) |
        # MLP2(t0) | MLP1(t1) | MLP2(t1).  Tile t1's DVE/ACT-bound LayerNorm
        # chains hide under tile t0's PE-bound MLP matmul stream.
        ps_s = ctx.enter_context(tc.tile_pool(name="ps_s", bufs=1, space="PSUM"))
        ps_q = ctx.enter_context(tc.tile_pool(name="ps_q", bufs=1, space="PSUM"))
        ps_fft = ctx.enter_context(tc.tile_pool(name="ps_fft", bufs=2, space="PSUM"))
        ps_mlp = ctx.enter_context(tc.tile_pool(name="ps_mlp", bufs=2, space="PSUM"))
        ps_out = ctx.enter_context(tc.tile_pool(name="ps_out", bufs=2, space="PSUM"))
        wblk = ctx.enter_context(tc.tile_pool(name="wblk", bufs=1))

        cm_fcos = tc.tile_pool(name="p_fcos", bufs=1, side="right")
        p_fcos = cm_fcos.__enter__()
        cm_xhf = [tc.tile_pool(name=f"p_xhf{t}", bufs=1, side="right")
                  for t in range(NT)]
        # open xhf1 BEFORE xhf0 so the right-side stack pops LIFO:
        # xhf0 (after phase1 t0), then xhf1, then fcos.
        p_xhf = [None, None]
        p_xhf[1] = cm_xhf[1].__enter__()
        p_xhf[0] = cm_xhf[0].__enter__()
        cm_m = [tc.tile_pool(name=f"p_m{t}", bufs=1) for t in range(NT)]

        x16_sb = [p_xhf[t].tile([P, KO, TT], F16, name=f"x16_{t}") for t in range(NT)]
        f_sb = [p_xhf[t].tile([P, KO, TT], F16, name=f"f{t}") for t in range(NT)]
        fcos_sb = p_fcos.tile([P, KO, 5 * P], F16)
        mir_sb = persist.tile([P, 2, P], F16)
        m_sb = [None, None]

        for t in range(NT):
            for k in range(KO):
                nc.sync.dma_start(
                    x16_sb[t][:, k, :], xT16_r[:, k, t * TT : (t + 1) * TT]
                )
        nc.sync.dma_start(par_sb, params[:, :])
        for k in range(KO):
            nc.sync.dma_start(fcos_sb[:, k, :], fcos_r[:, k, :])
        nc.sync.dma_start(mir_sb, mir.rearrange("two q p -> q two p"))

        def fft(t, mu16, rstd16, murstd16):
            # raw = x16 @ (g*COS); f = rstd*raw - (mu*rstd)*gcs + bfx
            # (LN1 folded into the weights; matmuls depend only on x16).
            for pair, ms in enumerate([(0, 1), (2, 3), (4,)]):
                psums = [
                    ps_fft.tile([P, TT], F32, tag="fft", name=f"fft{j}")
                    for j in range(len(ms))
                ]
                for k in range(KO):
                    for j, m in enumerate(ms):
                        nc.tensor.matmul(
                            psums[j],
                            lhsT=fcos_sb[:, k, m * P : (m + 1) * P],
                            rhs=x16_sb[t][:, k, :],
                            start=(k == 0), stop=(k == KO - 1),
                        )
                for j, m in enumerate(ms):
                    q1 = tmp.tile([P, TT], F16, tag="fq")
                    nc.vector.tensor_tensor(q1, psums[j], rstd16, ALU.mult)
                    u = tmp.tile([P, TT], F16, tag="fu")
                    nc.vector.tensor_scalar(
                        u, murstd16,
                        par_sb[:, _GCS_COL + m : _GCS_COL + m + 1],
                        par_sb[:, _BFX_COL + m : _BFX_COL + m + 1],
                        ALU.mult, ALU.subtract,
                    )
                    nc.vector.tensor_tensor(
                        f_sb[t][:, m, :], q1, u, ALU.subtract
                    )
            for m in (5, 6, 7):
                psum_m_ = ps_fft.tile([P, TT], F32, tag="fft", name="fftm")
                nc.tensor.matmul(
                    psum_m_, lhsT=mir_sb[:, 0, :], rhs=f_sb[t][:, 7 - m, :],
                    start=True, stop=False,
                )
                nc.tensor.matmul(
                    psum_m_, lhsT=mir_sb[:, 1, :], rhs=f_sb[t][:, 8 - m, :],
                    start=False, stop=True,
                )
                nc.scalar.activation(f_sb[t][:, m, :], psum_m_, AF.Copy)

        def lnf_stats(t, mu16, rstd16):
            """stats of f: mean(f) == LN1(x)[0] == g0*(x0-mu)*rstd + b0."""
            psum_s = ps_s.tile([P, TT], F32, tag="ps_s")
            psum_q = ps_q.tile([P, TT], F32, tag="ps_q")
            nc.tensor.matmul(
                psum_s, lhsT=ones_h[0:1, :], rhs=x16_sb[t][0:1, 0, :],
                start=True, stop=True,
            )
            for k in range(KO):
                sq = tmp.tile([P, TT], F16, tag="sq")
                nc.vector.tensor_mul(sq, f_sb[t][:, k, :], f_sb[t][:, k, :])
                nc.tensor.matmul(
                    psum_q, lhsT=ones_h, rhs=sq,
                    start=(k == 0), stop=(k == KO - 1),
                )
            s1 = stat.tile([P, TT], F16, tag="mu")  # becomes muf16
            nc.vector.tensor_tensor(s1, psum_s, mu16, ALU.subtract)
            nc.vector.tensor_tensor(s1, s1, rstd16, ALU.mult)
            nc.vector.tensor_scalar(
                s1, s1,
                par_sb[:, _G0_COL : _G0_COL + 1],
                par_sb[:, _B0_COL : _B0_COL + 1],
                ALU.mult, ALU.add,
            )
            musq = stat.tile([P, TT], F32, tag="musq")
            nc.scalar.activation(musq, s1, AF.Square)
            var = stat.tile([P, TT], F32, tag="var")
            nc.scalar.activation(var, psum_q, AF.Copy, scale=1.0 / C)
            nc.vector.tensor_tensor(var, var, musq, ALU.subtract)
            nc.scalar.activation(var, var, AF.Sqrt, bias=eps_sb)
            rstd = stat.tile([P, TT], F32, tag="rstd")
            nc.vector.reciprocal_approx_fast(rstd, var)
            rstdf16 = stat.tile([P, TT], F16, tag="rstd16")
            nc.vector.tensor_copy(rstdf16, rstd)
            return s1, rstdf16

        def lnf_residual_chunk(t, muf16, rstdf16, k):
            fn = tmp.tile([P, TT], F16, tag="fn")
            nc.vector.tensor_tensor(fn, f_sb[t][:, k, :], muf16, ALU.subtract)
            nc.vector.tensor_tensor(fn, fn, rstdf16, ALU.mult)
            fn16 = tmp.tile([P, TT], F16, tag="fn16")
            nc.vector.tensor_scalar(
                fn16, fn, pcol("lnf_g", k), pcol("lnf_b", k),
                ALU.mult, ALU.add,
            )
            nc.vector.tensor_tensor(
                x2_sb[t][:, k, :], x16_sb[t][:, k, :], fn16, ALU.add
            )

        def lnf_residual(t, muf16, rstdf16):
            for k in range(KO):
                lnf_residual_chunk(t, muf16, rstdf16, k)


        def mlp1(t, h_range):
            for h in h_range:
                w1blk = wblk.tile([P, KO, P], F16, tag="w1blk", bufs=3)
                nc.sync.dma_start(w1blk, w1b[h])
                psum_m = ps_mlp.tile([P, TT], F32, tag="mlp1")
                for k in range(KO):
                    nc.tensor.matmul(
                        psum_m, lhsT=w1blk[:, k, :], rhs=h2_sb[t][:, k, :],
                        start=(k == 0), stop=(k == KO - 1),
                    )
                nc.scalar.activation(
                    m_sb[t][:, h, :], psum_m, AF.Gelu,
                    bias=par_sb[:, _B1_COL + h : _B1_COL + h + 1],
                )

        def mlp2(t):
            for c in range(KO):
                w2blk = wblk.tile([P, HO, P], F16, tag="w2blk", bufs=2)
                nc.gpsimd.dma_start(w2blk, w2b[c])
                psum_o = ps_out.tile([P, TT], F32, tag="out")
                for h in range(HO):
                    nc.tensor.matmul(
                        psum_o,
                        lhsT=w2blk[:, h, :],
                        rhs=m_sb[t][:, h, :],
                        start=(h == 0), stop=(h == HO - 1),
                    )
                ob = outp.tile([P, TT], F32R, tag="ob")
                nc.scalar.activation(ob, psum_o, AF.Identity, bias=pcol("b2", c))
                nc.vector.tensor_tensor(ob, ob, x2_sb[t][:, c, :], ALU.add)
                nc.sync.dma_start(outT_r[:, c, t * TT : (t + 1) * TT], ob)

        # ---- tile 0 phase 1, tile-1 work woven in as PE filler ----
        st1_0 = ln_stats(x16_sb[0], ones_h, ps_s, ps_q)
        mrs0 = stat.tile([P, TT], F16, tag="mrs", name="mrs0", bufs=2)
        nc.vector.tensor_tensor(mrs0, st1_0[0], st1_0[1], ALU.mult)
        fft(0, *st1_0, mrs0)
        st1_1 = ln_stats(x16_sb[1], ones_h, ps_s, ps_q)
        mrs1 = stat.tile([P, TT], F16, tag="mrs", name="mrs1", bufs=2)
        nc.vector.tensor_tensor(mrs1, st1_1[0], st1_1[1], ALU.mult)
        fft(1, *st1_1, mrs1)
        stf0 = lnf_stats(0, *st1_0)
        lnf_residual(0, *stf0)
        st2_0 = ln_stats(x2_sb[0], ones_h, ps_s, ps_q)
        ln_apply(x2_sb[0], *st2_0, "ln2_g", "ln2_b", h2_sb[0])

        # ---- pipeline ----
        cm_xhf[0].__exit__(None, None, None)
        m_sb[0] = cm_m[0].__enter__().tile([P, HO, TT], F16, name="m0")

        mlp1(0, range(HO))

        stf1 = lnf_stats(1, *st1_1)
        lnf_residual(1, *stf1)
        st2_1 = ln_stats(x2_sb[1], ones_h, ps_s, ps_q)
        ln_apply(x2_sb[1], *st2_1, "ln2_g", "ln2_b", h2_sb[1])
        cm_xhf[1].__exit__(None, None, None)
        cm_fcos.__exit__(None, None, None)
        m_sb[1] = cm_m[1].__enter__().tile([P, HO, TT], F16, name="m1")

        mlp2(0)
        mlp1(1, range(HO))
        mlp2(1)

        cm_m[1].__exit__(None, None, None)
        cm_m[0].__exit__(None, None, None)

    nc.compile()
    return nc


_NC_CACHE: list = []


def _get_nc():
    if not _NC_CACHE:
        _NC_CACHE.append(_build_nc())
    return _NC_CACHE[0]


def _pack_params(inputs):
    p = np.zeros((P, _PWIDTH), np.float32)
    for name, col in _PCOLS.items():
        p[:, col : col + 8] = np.asarray(inputs[name], np.float32).reshape(8, P).T
    p[:, _B1_COL : _B1_COL + HO] = (
        np.asarray(inputs["b1"], np.float32).reshape(HO, P).T
    )
    n = np.arange(C, dtype=np.float64)
    cosm = np.cos((np.outer(n, n[: 5 * P]) % C) * (2.0 * np.pi / C))
    g1 = np.asarray(inputs["ln1_g"], np.float64)
    b1v = np.asarray(inputs["ln1_b"], np.float64)
    gcs = (g1[:, None] * cosm).sum(axis=0)          # [640]
    bfx = (b1v[:, None] * cosm).sum(axis=0)         # [640]
    p[:, _GCS_COL : _GCS_COL + 5] = gcs.reshape(5, P).T
    p[:, _BFX_COL : _BFX_COL + 5] = bfx.reshape(5, P).T
    p[:, _G0_COL] = np.float32(g1[0])
    p[:, _B0_COL] = np.float32(b1v[0])
    return p


def _run(inputs, trace=False):
    x = np.asarray(inputs["x"], np.float32)
    B, N, Cc = x.shape
    assert (B * N, Cc) == (N_CORES * TOK, C)
    x2d = x.reshape(B * N, C)

    n = np.arange(C, dtype=np.float64)
    # only the first 5*P output columns are computed directly (f[k] = f[C-k]);
    # LN1's per-channel gain is folded into the DFT matrix, its bias into a
    # per-output-channel additive term (see _pack_params).
    cosm = np.cos((np.outer(n, n[: 5 * P]) % C) * (2.0 * np.pi / C))
    g1 = np.asarray(inputs["ln1_g"], np.float64)
    fcos = (g1[:, None] * cosm).astype(np.float16)

    w1 = np.asarray(inputs["w1"], np.float32).astype(np.float16)
    w2 = np.asarray(inputs["w2"], np.float32).astype(np.float16)
    # block-contiguous layouts so each SBUF weight block is one clean DMA:
    # w1b[h, kp, ko, hc] = w1[ko*P+kp, h*P+hc]; w2b[c, hp, ho, cc] = w2[ho*P+hp, c*P+cc]
    w1bl = np.ascontiguousarray(
        w1.reshape(KO, P, HO, P).transpose(2, 1, 0, 3)
    )
    w2bl = np.ascontiguousarray(
        w2.reshape(HO, P, KO, P).transpose(2, 1, 0, 3)
    )
    # mirror matrices: out[p,t] = f7m[P-p, t] (p>=1);  out[0,t] = f8m[0, t]
    mirm = np.zeros((2, P, P), np.float16)
    for p_ in range(1, P):
        mirm[0, P - p_, p_] = 1.0
    mirm[1, 0, 0] = 1.0
    params = _pack_params(inputs)

    in_maps = []
    for i in range(N_CORES):
        shard = x2d[i * TOK : (i + 1) * TOK, :]
        in_maps.append(
            {
                "xT16": np.ascontiguousarray(shard.T).astype(np.float16),
                "fcos": fcos,
                "w1b": w1bl,
                "w2b": w2bl,
                "mir": mirm,
                "params": params,
            }
        )

    nc = _get_nc()
    res = run_bass_kernel_spmd(nc, in_maps, core_ids=list(range(N_CORES)), trace=trace)

    out2d = np.empty((B * N, C), np.float32)
    for i in range(N_CORES):
        out2d[i * TOK : (i + 1) * TOK, :] = res.results[i]["outT"].T
    return out2d.reshape(B, N, C), res


def kernel(**inputs) -> np.ndarray:
    return _run(inputs)[0]

